# revision 12
# baseline (speedup 1.0000x reference)
"""Trainium2 Bass kernel for an equivariant GNN message-passing layer.

Full inputs in, full output out. 8-way owner-computes sharding by edge target
node (col); windowed one-hot matmul segment-sum; per-edge source features
fetched with the MoE dma_gather primitive from a precomputed bf16 node table.

  reference math:
    dist  = |pos[row] - pos[col]|^2                                  [E,1]
    msg   = relu(cat(emb[row], emb[col], dist) @ W_msg + b_msg)      [E,H]
    aggr  = segment_sum(msg, col, N)                                 [N,H]
    out   = emb @ W_res + relu(cat(emb, aggr) @ W_upd + b_upd)       [N,H]

  device decomposition (per core, nodes [c*S, (c+1)*S)):
    A'[n] = emb[n] @ W1 + |pos[n]|^2 * w_d + b_msg   (bf16, + fp32 pos)
    B'[n] = emb[n] @ W2 + |pos[n]|^2 * w_d           (fp32, + fp32 pos)
    msg[e] = relu(A'[row_e] + B'[col_e] - 2*(pos_row . pos_col)*w_d)
    with W1 = W_msg[:128], W2 = W_msg[128:256], w_d = W_msg[256]

  dma_gather needs int16 indices, so the A' table is gathered in two halves
  (rows < N_pad/2 and >= N_pad/2); every 128-edge tile is half-pure and
  consecutive same-half tiles form one gather "run" (<= RMAX tiles).
"""

import sys

for _p in ("/opt/trn_rl_repo",):
    if _p not in sys.path:
        sys.path.insert(0, _p)

import numpy as np
import ml_dtypes

import concourse.bacc as bacc
import concourse.bass as bass
import concourse.mybir as mybir
import concourse.tile as tile
from concourse.bass_utils import run_bass_kernel_spmd

F32 = mybir.dt.float32
BF16 = mybir.dt.bfloat16
I16 = mybir.dt.int16

H = 128          # hidden/in channels (hardcoded for this problem)
AW = 256         # A' table row width in bf16 elems (512B rows)
BW = 136         # B' table row width (fp32)
RMAX = 8         # max 128-edge tiles per gather run


# --------------------------------------------------------------------------
# host-side prep
# --------------------------------------------------------------------------

def host_prep(node_embed, node_pos, W_res, W_msg, b_msg, W_upd, b_upd,
              edge_index, n_cores):
    N, C = node_embed.shape
    assert C == H and W_msg.shape == (2 * H + 1, H)
    assert N % n_cores == 0
    S = N // n_cores
    n_win = -(-S // 128)
    S_pad = n_win * 128
    N_pad = -(-N // 256) * 256          # even number of 128-blocks
    N_half = N_pad // 2

    row = np.asarray(edge_index[0], dtype=np.int64)
    col = np.asarray(edge_index[1], dtype=np.int64)
    core_of = col // S

    # per-core edge lists sorted by (window, half)
    per_core = []
    counts = np.zeros((n_cores, n_win, 2), dtype=np.int64)
    for c in range(n_cores):
        sel = np.nonzero(core_of == c)[0]
        lc = col[sel] - c * S
        w = lc // 128
        hf = (row[sel] >= N_half).astype(np.int64)
        order = np.lexsort((hf, w))
        sel, w, hf = sel[order], w[order], hf[order]
        cw = (lc[order] % 128).astype(np.float32)
        np.add.at(counts[c], (w, hf), 1)
        per_core.append((sel, cw))

    tiles_wh = -(-counts.max(axis=0) // 128)            # [n_win, 2] tile counts
    win_of_tile, half_of_tile = [], []
    for w in range(n_win):
        for hf in (0, 1):
            win_of_tile += [w] * int(tiles_wh[w, hf])
            half_of_tile += [hf] * int(tiles_wh[w, hf])
    T = len(win_of_tile)
    first_tile, last_tile = {}, {}
    for t, w in enumerate(win_of_tile):
        first_tile.setdefault(w, t)
        last_tile[w] = t

    # gather runs: maximal same-half tile sequences, capped at RMAX
    runs = []            # (tile0, L, half)
    t = 0
    while t < T:
        hf = half_of_tile[t]
        L = 1
        while (t + L < T and half_of_tile[t + L] == hf and L < RMAX):
            L += 1
        runs.append((t, L, hf))
        t += L
    R = len(runs)

    tile_base = {}
    b = 0
    for w in range(n_win):
        for hf in (0, 1):
            tile_base[(w, hf)] = b
            b += int(tiles_wh[w, hf])

    # per-core index arrays
    gidx_all, colp_all, colf_all = [], [], []
    for c in range(n_cores):
        sel, cw = per_core[c]
        rows16 = np.zeros((T, 128), dtype=np.int16)     # per (tile, lane)
        colp = np.full((128, T), -1.0, dtype=np.float32)
        colf = np.full((R, RMAX * 128), -1.0, dtype=np.float32)
        start = 0
        for w in range(n_win):
            for hf in (0, 1):
                cnt = int(counts[c, w, hf])
                if cnt:
                    idx = np.arange(cnt)
                    t_loc = tile_base[(w, hf)] + idx // 128
                    lane = idx % 128
                    r = row[sel[start:start + cnt]] - hf * N_half
                    rows16[t_loc, lane] = r.astype(np.int16)
                    start += cnt
                    colp[lane, t_loc] = cw[start - cnt:start]
        gidx16 = np.zeros((R, 128, RMAX * 8), dtype=np.int16)
        for ri, (t0, L, hf) in enumerate(runs):
            flat = rows16[t0:t0 + L].reshape(L * 128)        # j = k*128+lane
            wrap = flat.reshape(-1, 16).T                    # [16, L*8]
            for rep in range(8):
                gidx16[ri, rep * 16:(rep + 1) * 16, :L * 8] = wrap
            for k in range(L):
                colf[ri, k * 128:(k + 1) * 128] = colp[:, t0 + k]
        gidx_all.append(gidx16)
        colp_all.append(colp)
        colf_all.append(colf)

    # replicated tensors
    bf = ml_dtypes.bfloat16
    emb = np.asarray(node_embed, dtype=np.float32)
    embT_full = np.zeros((H, N_pad), dtype=bf)
    embT_full[:, :N] = emb.T.astype(bf)
    pos_full = np.zeros((N_pad, 3), dtype=np.float32)
    pos_full[:N] = np.asarray(node_pos, dtype=np.float32)

    iota = np.arange(128, dtype=np.float32)
    W_msg = np.asarray(W_msg, dtype=np.float32)
    wdiag = np.zeros((RMAX, RMAX * 128), dtype=np.float32)
    for k in range(RMAX):
        wdiag[k, k * 128:(k + 1) * 128] = W_msg[2 * H]
    repl = {
        "embT_full": embT_full,
        "pos_full": pos_full,
        "W1": np.ascontiguousarray(W_msg[:H]).astype(bf),
        "W2": np.ascontiguousarray(W_msg[H:2 * H]).astype(bf),
        "wd_rep": np.tile(W_msg[2 * H:2 * H + 1], (128, 1)),
        "wdiag": wdiag,
        "bmsg_row": np.asarray(b_msg, dtype=np.float32).reshape(1, H),
        "W_res": np.asarray(W_res, dtype=np.float32),
        "Wu1": np.ascontiguousarray(np.asarray(W_upd, dtype=np.float32)[:H]),
        "Wu2": np.ascontiguousarray(np.asarray(W_upd, dtype=np.float32)[H:]),
        "bupd_col": np.asarray(b_upd, dtype=np.float32).reshape(H, 1),
        "identity": np.eye(128, dtype=np.float32),
        "ident_bf": np.eye(128, dtype=bf),
        "iota_p": iota.reshape(128, 1).copy(),
        "iota_rep": np.tile(iota.reshape(1, 128), (128, 1)),
        "ones_row": np.ones((1, 128), np.float32),
    }

    in_maps = []
    for c in range(n_cores):
        shardT = np.zeros((H, S_pad), dtype=bf)
        shardT[:, :S] = emb[c * S:(c + 1) * S].T.astype(bf)
        pos_shard = np.zeros((S_pad, 3), dtype=np.float32)
        pos_shard[:S] = np.asarray(node_pos, dtype=np.float32)[c * S:(c + 1) * S]
        m = dict(repl)
        m["emb_shardT"] = shardT
        m["pos_shard"] = pos_shard
        m["gidx16"] = gidx_all[c]
        m["colp"] = colp_all[c]
        m["colf"] = colf_all[c]
        in_maps.append(m)

    cfg = dict(N=N, N_pad=N_pad, N_half=N_half, S=S, S_pad=S_pad, n_win=n_win,
               R=R, T=T, runs=runs, win_of_tile=win_of_tile,
               first_tile=first_tile, last_tile=last_tile, n_cores=n_cores)
    return cfg, in_maps


# --------------------------------------------------------------------------
# device program
# --------------------------------------------------------------------------

def build_program(cfg, debug=False):
    N_pad, N_half, S_pad, n_win, R, T = (cfg["N_pad"], cfg["N_half"],
                                         cfg["S_pad"], cfg["n_win"],
                                         cfg["R"], cfg["T"])
    runs = cfg["runs"]
    win_of_tile = cfg["win_of_tile"]
    first_tile, last_tile = cfg["first_tile"], cfg["last_tile"]

    nc = bacc.Bacc("TRN2", target_bir_lowering=False, debug=debug,
                   num_devices=cfg["n_cores"])

    din = lambda n, s, dt=F32: nc.dram_tensor(n, s, dt, kind="ExternalInput")
    embT_full = din("embT_full", [H, N_pad], BF16)
    pos_full = din("pos_full", [N_pad, 3])
    W1 = din("W1", [H, H], BF16)
    W2 = din("W2", [H, H], BF16)
    wd_rep = din("wd_rep", [128, H])
    wdiag = din("wdiag", [RMAX, RMAX * 128])
    bmsg_row = din("bmsg_row", [1, H])
    W_res = din("W_res", [H, H]); Wu1 = din("Wu1", [H, H]); Wu2 = din("Wu2", [H, H])
    bupd_col = din("bupd_col", [H, 1])
    identity = din("identity", [128, 128])
    ident_bf = din("ident_bf", [128, 128], BF16)
    iota_p = din("iota_p", [128, 1]); iota_rep = din("iota_rep", [128, 128])
    ones_row = din("ones_row", [1, 128])
    emb_shardT = din("emb_shardT", [H, S_pad], BF16)
    pos_shard = din("pos_shard", [S_pad, 3])
    gidx16 = din("gidx16", [R, 128, RMAX * 8], I16)
    colp = din("colp", [128, T])
    colf = din("colf", [R, RMAX * 128])

    At = nc.dram_tensor("At", [N_pad, AW], BF16)      # A' table (scratch)
    Bd = nc.dram_tensor("Bd", [S_pad, BW], F32)       # B' table (scratch)
    out_d = nc.dram_tensor("out", [S_pad, H], F32, kind="ExternalOutput")

    with tile.TileContext(nc) as tc:
        with (
            tc.tile_pool(name="const", bufs=1) as cp,
            tc.tile_pool(name="sb", bufs=2) as sb,
            tc.tile_pool(name="big", bufs=1) as bigp,
            tc.tile_pool(name="ps", bufs=2, space="PSUM") as ps,
            tc.tile_pool(name="ps1", bufs=1, space="PSUM") as ps1,
        ):
            def cload(t, shape, dt=F32):
                s = cp.tile(shape, dt, tag=t.name)
                nc.sync.dma_start(s[:], t[:])
                return s

            W1_sb = cload(W1, [H, H], BF16)
            W2_sb = cload(W2, [H, H], BF16)
            wd_rep_sb = cload(wd_rep, [128, H])
            wdiag_sb = cload(wdiag, [RMAX, RMAX * 128])
            bmsg_sb = cload(bmsg_row, [1, H])
            Wres_sb = cload(W_res, [H, H])
            Wu1_sb = cload(Wu1, [H, H])
            Wu2_sb = cload(Wu2, [H, H])
            bupd_sb = cload(bupd_col, [H, 1])
            ident_sb = cload(identity, [128, 128])
            ident_bf_sb = cload(ident_bf, [128, 128], BF16)
            iota_p_sb = cload(iota_p, [128, 1])
            iota_rep_sb = cload(iota_rep, [128, 128])
            ones_sb = cload(ones_row, [1, 128])
            colp_sb = cp.tile([128, T], F32, tag="colp")
            nc.sync.dma_start(colp_sb[:], colp[:])
            aggrT_sb = bigp.tile([128, S_pad], F32, tag="aggrT")

            # ---------------- P1: build A' (bf16) and B' (fp32) tables -----
            def build_table(dst, dstw, dst_dt, srcT, srcPos, n_rows, Wmat,
                            with_bias):
                pos_hi = 134 if dst_dt == BF16 else 131
                n_chunks = n_rows // 128
                for q0 in range(0, n_chunks, 4):
                    nj = min(4, n_chunks - q0)
                    psA = ps.tile([128, 4, 128], F32, tag="small_ps")
                    stage = sb.tile([128, 4, dstw], dst_dt,
                                    tag="stageA" if dst_dt == BF16 else "stageB")
                    posw = (stage[:, 0:nj, 128:134].bitcast(F32)
                            if dst_dt == BF16 else stage[:, 0:nj, 128:131])
                    nc.sync.dma_start(
                        out=posw,
                        in_=srcPos[q0 * 128:(q0 + nj) * 128, :]
                        .rearrange("(j p) d -> p j d", p=128),
                    )
                    for j in range(nj):
                        c = q0 + j
                        embT_c = sb.tile([H, 128], BF16, tag="embT_c")
                        nc.sync.dma_start(embT_c[:], srcT[:, c * 128:(c + 1) * 128])
                        nc.tensor.matmul(out=psA[:, j, :], lhsT=embT_c[:],
                                         rhs=Wmat[:], start=True,
                                         stop=not with_bias)
                        if with_bias:
                            nc.tensor.matmul(out=psA[:, j, :], lhsT=ones_sb[:],
                                             rhs=bmsg_sb[:], start=False,
                                             stop=True)
                    prod = sb.tile([128, 4, 3], F32, tag="p1prod")
                    nc.vector.tensor_tensor(out=prod[:, 0:nj, :],
                                            in0=posw, in1=posw,
                                            op=mybir.AluOpType.mult)
                    sq = sb.tile([128, 4, 1], F32, tag="p1sq")
                    nc.vector.tensor_reduce(out=sq[:, 0:nj, :],
                                            in_=prod[:, 0:nj, :],
                                            axis=mybir.AxisListType.X,
                                            op=mybir.AluOpType.add)
                    for j in range(nj):
                        nc.vector.scalar_tensor_tensor(
                            out=stage[:, j, 0:128], in0=wd_rep_sb[:],
                            scalar=sq[:, j, :], in1=psA[:, j, :],
                            op0=mybir.AluOpType.mult, op1=mybir.AluOpType.add)
                    nc.vector.memset(stage[:, 0:nj, pos_hi:dstw], 0.0)
                    nc.sync.dma_start(
                        out=dst[q0 * 128:(q0 + nj) * 128, :]
                        .rearrange("(j p) f -> p j f", p=128),
                        in_=stage[:, 0:nj, :],
                    )

            build_table(At, AW, BF16, embT_full, pos_full, N_pad, W1_sb, True)
            build_table(Bd, BW, F32, emb_shardT, pos_shard, S_pad, W2_sb, False)

            # ---------------- P2: edge loop over gather runs ---------------
            Bres = bigp.tile([128, n_win, 131], F32, tag="Bres")
            nc.sync.dma_start(
                Bres[:], Bd[:, 0:131].rearrange("(w p) f -> p w f", p=128))
            aggr_tiles = {}

            for ri, (t0, L, hf) in enumerate(runs):
                il = sb.tile([128, RMAX * 8], I16, tag="il")
                nc.sync.dma_start(il[:], gidx16[ri])
                Ag = sb.tile([128, RMAX, AW], BF16, tag="Ag")
                src = At[0:N_half, :] if hf == 0 else At[N_half:N_pad, :]
                nc.gpsimd.dma_gather(Ag[:, 0:L, :], src, il[:, 0:L * 8],
                                     L * 128, L * 128, AW)
                # one-hots
                colf_g = sb.tile([1, RMAX * 128], F32, tag="colf_g")
                nc.sync.dma_start(colf_g[:, 0:L * 128], colf[ri:ri + 1, 0:L * 128])
                msg_ps = ps.tile([128, RMAX, 128], F32, tag="msgps")
                flat = msg_ps[:].rearrange("p k e -> p (k e)")
                for o in range(0, L * 128, 512):
                    oe = min(o + 512, L * 128)
                    nc.tensor.matmul(out=flat[:, o:oe], lhsT=ones_sb[:],
                                     rhs=colf_g[:, o:oe], start=True, stop=True)
                oT = sb.tile([128, RMAX, 128], F32, tag="oT")
                nc.vector.tensor_tensor(
                    out=oT[:, 0:L, :],
                    in0=iota_p_sb[:, :, None].to_broadcast([128, L, 128]),
                    in1=msg_ps[:, 0:L, :], op=mybir.AluOpType.is_equal)
                o8 = sb.tile([128, RMAX, 128], F32, tag="o8")
                nc.vector.tensor_tensor(
                    out=o8[:, 0:L, :],
                    in0=colp_sb[:, t0:t0 + L, None].to_broadcast([128, L, 128]),
                    in1=iota_rep_sb[:, None, :].to_broadcast([128, L, 128]),
                    op=mybir.AluOpType.is_equal)
                # p_col then cross = -2*(p_row . p_col)
                pcol_ps = ps.tile([128, RMAX, 4], F32, tag="small_ps")
                for k in range(L):
                    w = win_of_tile[t0 + k]
                    nc.tensor.matmul(out=pcol_ps[:, k, 0:3], lhsT=oT[:, k, :],
                                     rhs=Bres[:, w, 128:131], start=True,
                                     stop=True)
                prod = sb.tile([128, RMAX, 3], F32, tag="prod")
                nc.vector.tensor_tensor(out=prod[:, 0:L, :],
                                        in0=Ag[:, 0:L, 128:134].bitcast(F32),
                                        in1=pcol_ps[:, 0:L, 0:3],
                                        op=mybir.AluOpType.mult)
                cross = sb.tile([128, RMAX, 1], F32, tag="cross")
                nc.vector.tensor_reduce(out=cross[:, 0:L, :],
                                        in_=prod[:, 0:L, :],
                                        axis=mybir.AxisListType.X,
                                        op=mybir.AluOpType.add)
                crossT_ps = ps1.tile([RMAX, 128], F32, tag="ctps")
                nc.tensor.transpose(out=crossT_ps[0:L, :],
                                    in_=cross[:, 0:L, 0],
                                    identity=ident_sb[:])
                crossT_sb = sb.tile([RMAX, 128], F32, tag="ct")
                nc.scalar.activation(out=crossT_sb[0:L, :],
                                     in_=crossT_ps[0:L, :],
                                     func=mybir.ActivationFunctionType.Copy,
                                     scale=-2.0)
                # message pre-activation: dist term first (start), then
                # per-tile B'-broadcast and A'-gather accumulates
                for o in range(0, L * 128, 512):
                    oe = min(o + 512, L * 128)
                    nc.tensor.matmul(out=flat[:, o:oe], lhsT=crossT_sb[0:L, :],
                                     rhs=wdiag_sb[0:L, o:oe], start=True,
                                     stop=False, skip_group_check=True)
                for k in range(L):
                    w = win_of_tile[t0 + k]
                    nc.tensor.matmul(out=msg_ps[:, k, :], lhsT=oT[:, k, :],
                                     rhs=Bres[:, w, 0:128], start=False,
                                     stop=False, skip_group_check=True)
                    nc.tensor.matmul(out=msg_ps[:, k, :], lhsT=ident_bf_sb[:],
                                     rhs=Ag[:, k, 0:128], start=False,
                                     stop=True, skip_group_check=True)
                msg_sb = sb.tile([128, RMAX, 128], F32, tag="msg")
                for o in range(0, L, 4):
                    oe = min(o + 4, L)
                    nc.scalar.activation(out=msg_sb[:, o:oe, :],
                                         in_=msg_ps[:, o:oe, :],
                                         func=mybir.ActivationFunctionType.Relu)
                # segment sum into per-window aggr psum
                for k in range(L):
                    t = t0 + k
                    w = win_of_tile[t]
                    if t == first_tile[w]:
                        aggr_t = ps1.tile([128, 128], F32, tag="aggr")
                        aggr_tiles[w] = aggr_t
                    nc.tensor.matmul(out=aggr_tiles[w][:], lhsT=msg_sb[:, k, :],
                                     rhs=o8[:, k, :],
                                     start=(t == first_tile[w]),
                                     stop=(t == last_tile[w]))
                    if t == last_tile[w]:
                        nc.scalar.activation(
                            out=aggrT_sb[:, w * 128:(w + 1) * 128],
                            in_=aggr_tiles[w][:],
                            func=mybir.ActivationFunctionType.Copy)
                        del aggr_tiles[w]

            # ---------------- P3: node update MLP --------------------------
            for c in range(S_pad // 128):
                embT_c3 = sb.tile([H, 128], BF16, tag="embT_c")
                nc.sync.dma_start(embT_c3[:], emb_shardT[:, c * 128:(c + 1) * 128])
                emb32 = sb.tile([H, 128], F32, tag="emb32")
                nc.vector.tensor_copy(out=emb32[:], in_=embT_c3[:])
                ps_r = ps.tile([128, 128], F32, tag="small_ps")
                nc.tensor.matmul(out=ps_r[:], lhsT=Wres_sb[:], rhs=emb32[:],
                                 start=True, stop=True)
                ps_u = ps.tile([128, 128], F32, tag="small_ps")
                nc.tensor.matmul(out=ps_u[:], lhsT=Wu1_sb[:], rhs=emb32[:],
                                 start=True, stop=False)
                nc.tensor.matmul(out=ps_u[:], lhsT=Wu2_sb[:],
                                 rhs=aggrT_sb[:, c * 128:(c + 1) * 128],
                                 start=False, stop=True)
                r_sb = sb.tile([128, 128], F32, tag="r3")
                nc.scalar.activation(out=r_sb[:], in_=ps_u[:],
                                     func=mybir.ActivationFunctionType.Relu,
                                     bias=bupd_sb[:])
                outT_sb = sb.tile([128, 128], F32, tag="o3")
                nc.vector.tensor_tensor(out=outT_sb[:], in0=ps_r[:], in1=r_sb[:],
                                        op=mybir.AluOpType.add)
                ps_t = ps1.tile([128, 128], F32, tag="aggr")
                nc.tensor.transpose(out=ps_t[:], in_=outT_sb[:],
                                    identity=ident_sb[:])
                out_sb = sb.tile([128, 128], F32, tag="out3")
                nc.scalar.activation(out=out_sb[:], in_=ps_t[:],
                                     func=mybir.ActivationFunctionType.Copy)
                nc.sync.dma_start(out_d[c * 128:(c + 1) * 128, :], out_sb[:])

    nc.compile()
    return nc


# --------------------------------------------------------------------------
# entry point
# --------------------------------------------------------------------------

def kernel(node_embed, node_pos, W_res, W_msg, b_msg, W_upd, b_upd,
           edge_index, n_cores=8, _run=None):
    cfg, in_maps = host_prep(node_embed, node_pos, W_res, W_msg, b_msg,
                             W_upd, b_upd, edge_index, n_cores)
    nc = build_program(cfg)
    if _run is None:
        res = run_bass_kernel_spmd(nc, in_maps, core_ids=list(range(n_cores)))
        outs = [res.results[c]["out"] for c in range(n_cores)]
    else:
        outs = _run(nc, in_maps)
    S = cfg["S"]
    return np.concatenate([o[:S] for o in outs], axis=0)
